# revision 15
# baseline (speedup 1.0000x reference)
"""Multi-head attention (B=4, S=2048, D=512, H=8) on 8 trn2 NeuronCores.

Sharding: core c = (batch b = c//2, query-half qh = c%2). Each core computes
the full attention output for 1024 query rows of one batch element.

Device-side scheme (all layouts chosen so no on-chip transposes are needed):
  - host supplies transposed activations (X^T = [D, S] layouts)
  - Q^T, K^T projections: lhsT = weight chunk, rhs = X^T chunk; per-partition
    bias added during the PSUM->SBUF drain.
  - V projection in natural [s, dout] layout (lhsT = X^T chunk, rhs = wv),
    stored as [128, H, DH+1] with a ones column per head (denominator trick).
  - logits computed transposed: lg^T[s_k, q] = K_h^T-block . Q_h^T, two heads
    per PE pass (row-packed at partitions 0-63 / 64-127, K=64 each).
  - exp on ACT engine with per-partition mask bias: w = exp(0.125*lg + mb).
    No max-subtraction: logits ~ N(0,1) here, exp is safe in fp32, and
    masked/padded keys get bias -1e9 -> exp exactly 0 (matches reference).
  - AV: attnU^T[d, q] accumulated col-packed (head A -> psum rows 0-63,
    head B -> rows 64-127 of a second tile; concurrent on the PE array)
    + m=1 denominator matmuls against the ones column.
  - normalization: r = 1/denom (DVE), partition-broadcast via K=1 PE outer
    products, two DVE multiplies.
  - O projection: out[q, dout] with lhsT = attnN^T (natural layout), K=1
    matmul adds the output bias row.

Masked keys (mask==1) are compacted away on the host: their softmax weight is
exactly 0 in the reference (exp underflows), so dropping them is exact and
roughly halves attention compute. Keys are padded to a multiple of 128 with
-1e9 mask bias.

Matmul operands are bitcast to float32r (full-rate fp32 path on the PE; plain
fp32 matmul runs at 1/4 rate).
"""

import os
import numpy as np

B, S, D, H = 4, 2048, 512, 8
DH = D // H
NCORE = 8
SQ = S // 2  # queries per core
SCALE = 1.0 / float(np.sqrt(DH))

_BUILT = {}


def _chunks(total, step):
    out = []
    c0 = 0
    while c0 < total:
        out.append((c0, min(step, total - c0)))
        c0 += step
    return out


def build_bass(s_pad, mm_dtype="bf16"):
    import concourse.bass as bass  # noqa: F401
    import concourse.mybir as mybir
    import concourse.tile as tile
    from concourse import bacc
    from contextlib import ExitStack

    f32 = mybir.dt.float32
    mmdt = {
        "bf16": mybir.dt.bfloat16,
        "f32r": mybir.dt.float32r,
        "f32": mybir.dt.float32,
    }[mm_dtype]
    f32r = mybir.dt.float32r
    EXP = mybir.ActivationFunctionType.Exp

    nsk = s_pad // 128

    nc = bacc.Bacc(
        "TRN2",
        target_bir_lowering=False,
        debug=False,
        enable_asserts=False,
        num_devices=NCORE,
    )

    KW, QW = 4 * s_pad, 4 * SQ
    d_bk_blob = nc.dram_tensor("blob_k", [128, 2048 + KW], mmdt, kind="ExternalInput").ap()
    d_bq_blob = nc.dram_tensor("blob_q", [128, 2048 + QW], mmdt, kind="ExternalInput").ap()
    d_bv_blob = nc.dram_tensor("blob_v", [128, 2048 + KW + D + 8], mmdt, kind="ExternalInput").ap()
    d_bo_blob = nc.dram_tensor("blob_o", [128, 2048 + D], mmdt, kind="ExternalInput").ap()
    d_mb = nc.dram_tensor("mb", [128, nsk], f32, kind="ExternalInput").ap()
    d_bq = nc.dram_tensor("bq_pp", [128, 4], f32, kind="ExternalInput").ap()
    d_bk = nc.dram_tensor("bk_pp", [128, 4], f32, kind="ExternalInput").ap()
    d_out = nc.dram_tensor("out", [SQ, D], f32, kind="ExternalOutput").ap()

    def r(ap):
        return ap

    with tile.TileContext(nc) as tc, ExitStack() as ctx, nc.allow_low_precision(
        "matmul operands held as float32r (full-rate PE fp32 path)"
    ):
        sb = ctx.enter_context(tc.tile_pool(name="sb", bufs=1))
        ps_lg = ctx.enter_context(tc.tile_pool(name="pslg", bufs=2, space="PSUM"))
        ps_av = ctx.enter_context(tc.tile_pool(name="psav", bufs=2, space="PSUM"))

        def load(pool, name, shape, src, dt=None):
            t = pool.tile(shape, mmdt if dt is None else dt, tag=name, name=name)
            nc.sync.dma_start(t[:], src)
            return t

        # projection outputs (persistent)
        kT = [sb.tile([128, s_pad], mmdt, tag=f"kT{j}", name=f"kT{j}") for j in range(4)]
        qT = [sb.tile([128, SQ], mmdt, tag=f"qT{j}", name=f"qT{j}") for j in range(4)]
        attnN = [
            sb.tile([128, SQ], mmdt, tag=f"attnN{pr}", name=f"attnN{pr}")
            for pr in range(4)
        ]
        v = [
            sb.tile([128, H, DH + 1], mmdt, tag=f"v{t}", name=f"v{t}")
            for t in range(nsk)
        ]

        # ---- projection phase (inputs in a scoped pool, freed afterwards) ----
        with tc.tile_pool(name="inp", bufs=1) as inp:
            # one full-bandwidth blob DMA per input group, in consumption order
            blk = load(inp, "blk", [128, 2048 + KW], d_bk_blob[:])
            bk = load(inp, "bk", [128, 4], d_bk[:], dt=f32)
            blq = load(inp, "blq", [128, 2048 + QW], d_bq_blob[:])
            bq = load(inp, "bq", [128, 4], d_bq[:], dt=f32)
            blv = load(inp, "blv", [128, 2048 + KW + D + 8], d_bv_blob[:])
            mb = load(sb, "mb", [128, nsk], d_mb[:], dt=f32)
            blo = load(sb, "blo", [128, 2048 + D], d_bo_blob[:])
            w_t = {
                "wk": [blk[:, dk * 512 : (dk + 1) * 512] for dk in range(4)],
                "wq": [blq[:, dk * 512 : (dk + 1) * 512] for dk in range(4)],
                "wv": [blv[:, dk * 512 : (dk + 1) * 512] for dk in range(4)],
            }
            xk = [blk[:, 2048 + dk * s_pad : 2048 + (dk + 1) * s_pad] for dk in range(4)]
            xq = [blq[:, 2048 + dk * SQ : 2048 + (dk + 1) * SQ] for dk in range(4)]
            xv = [blv[:, 2048 + dk * s_pad : 2048 + (dk + 1) * s_pad] for dk in range(4)]
            bvb = blv[:, 2048 + KW : 2048 + KW + D]
            ones8 = blv[:, 2048 + KW + D : 2048 + KW + D + 8]
            wo_t = [blo[:, dk * 512 : (dk + 1) * 512] for dk in range(4)]
            bob = blo[:, 2048 : 2048 + D]

            # K^T projection -> kT[j] [128, s_pad] (dout tile j; heads 2j, 2j+1)
            for j in range(4):
                for c0, cw in _chunks(s_pad, 512):
                    ps = ps_lg.tile([128, cw], f32, tag="lg", name="lg")
                    for dk in range(4):
                        nc.tensor.matmul(
                            ps[:],
                            lhsT=r(w_t["wk"][dk][:, j * 128 : (j + 1) * 128]),
                            rhs=r(xk[dk][:, c0 : c0 + cw]),
                            start=(dk == 0),
                            stop=(dk == 3),
                        )
                    nc.vector.tensor_scalar_add(
                        kT[j][:, c0 : c0 + cw], ps[:], bk[:, j : j + 1]
                    )
            # Q^T projection -> qT[j] [128, SQ]
            for j in range(4):
                for c0, cw in _chunks(SQ, 512):
                    ps = ps_lg.tile([128, cw], f32, tag="lg", name="lg")
                    for dk in range(4):
                        nc.tensor.matmul(
                            ps[:],
                            lhsT=r(w_t["wq"][dk][:, j * 128 : (j + 1) * 128]),
                            rhs=r(xq[dk][:, c0 : c0 + cw]),
                            start=(dk == 0),
                            stop=(dk == 3),
                        )
                    nc.vector.tensor_scalar_add(
                        qT[j][:, c0 : c0 + cw], ps[:], bq[:, j : j + 1]
                    )
            # V projection -> v[t] [128, H, DH+1] with ones column
            for t in range(nsk):
                ps = ps_lg.tile([128, D], f32, tag="lg", name="lg")
                for dk in range(4):
                    nc.tensor.matmul(
                        ps[:],
                        lhsT=r(xv[dk][:, t * 128 : (t + 1) * 128]),
                        rhs=r(w_t["wv"][dk]),
                        start=(dk == 0),
                        stop=(dk == 3),
                    )
                nc.vector.tensor_copy(
                    v[t][:, :, DH : DH + 1], ones8.rearrange("p (h o) -> p h o", o=1)
                )
                nc.vector.scalar_tensor_tensor(
                    v[t][:, :, 0:DH],
                    ps[:].rearrange("p (h d) -> p h d", h=H),
                    1.0,
                    bvb.rearrange("p (h d) -> p h d", h=H),
                    op0=mybir.AluOpType.mult,
                    op1=mybir.AluOpType.add,
                )

        # ---- attention phase ----
        wexp_p = ctx.enter_context(tc.tile_pool(name="wexp", bufs=4))
        osb_p = ctx.enter_context(tc.tile_pool(name="osb", bufs=2))
        r_p = ctx.enter_context(tc.tile_pool(name="rp", bufs=2))

        def sk_loop(qc, pr):
            q0 = qc * 512
            hA, hB = 2 * pr, 2 * pr + 1
            avA = ps_av.tile([65, 512], f32, tag="avA", name="avA")
            avB = ps_av.tile([65, 512], f32, tag="avB", name="avB")
            for t in range(nsk):
                lg = ps_lg.tile([128, 1024], f32, tag="lg", name="lg")
                nc.tensor.matmul(
                    lg[:, 0:512],
                    lhsT=r(kT[pr][0:64, t * 128 : (t + 1) * 128]),
                    rhs=r(qT[pr][0:64, q0 : q0 + 512]),
                    start=True,
                    stop=True,
                )
                nc.tensor.matmul(
                    lg[:, 512:1024],
                    lhsT=r(kT[pr][64:128, t * 128 : (t + 1) * 128]),
                    rhs=r(qT[pr][64:128, q0 : q0 + 512]),
                    start=True,
                    stop=True,
                )
                wx = wexp_p.tile([128, 1024], mmdt, tag="wexp", name="wexp")
                nc.scalar.activation(
                    wx[:], lg[:], EXP, bias=mb[:, t : t + 1], scale=SCALE
                )
                last = t == nsk - 1
                nc.tensor.matmul(
                    avA[0:65, :],
                    lhsT=r(v[t][:, hA : hA + 1, 0 : DH + 1]),
                    rhs=r(wx[:, 0:512]),
                    start=(t == 0),
                    stop=last,
                )
                nc.tensor.matmul(
                    avB[0:65, :],
                    lhsT=r(v[t][:, hB : hB + 1, 0 : DH + 1]),
                    rhs=r(wx[:, 512:1024]),
                    start=(t == 0),
                    stop=last,
                )
            # kick off the DVE reciprocal chain now; bc matmuls are emitted
            # later (pipelined) so the PE never waits on this chain
            dsA = r_p.tile([1, 512], f32, tag="dsA", name="dsA")
            dsB = r_p.tile([1, 512], f32, tag="dsB", name="dsB")
            nc.vector.tensor_copy(dsA[0:1, :], avA[64:65, :])
            nc.vector.tensor_copy(dsB[0:1, :], avB[64:65, :])
            rfA = r_p.tile([1, 512], f32, tag="rfA", name="rfA")
            rfB = r_p.tile([1, 512], f32, tag="rfB", name="rfB")
            nc.vector.reciprocal_approx_fast(rfA[0:1, :], dsA[0:1, :])
            nc.vector.reciprocal_approx_fast(rfB[0:1, :], dsB[0:1, :])
            bcsA = r_p.tile([64, 512], f32, tag="bcsA", name="bcsA")
            bcsB = r_p.tile([64, 512], f32, tag="bcsB", name="bcsB")
            nc.gpsimd.partition_broadcast(bcsA[0:64, :], rfA[0:1, :], channels=64)
            nc.gpsimd.partition_broadcast(bcsB[0:64, :], rfB[0:1, :], channels=64)
            return (qc, pr, avA, avB, bcsA, bcsB)

        def finish(qc, pr, avA, avB, bcsA, bcsB):
            q0 = qc * 512
            nc.vector.tensor_mul(
                attnN[pr][0:64, q0 : q0 + 512], avA[0:64, :], bcsA[0:64, :]
            )
            nc.vector.tensor_mul(
                attnN[pr][64:128, q0 : q0 + 512], avB[0:64, :], bcsB[0:64, :]
            )

        def o_proj(qc):
            q0 = qc * 512
            for qt in range(4):
                qq = q0 + qt * 128
                ops = ps_lg.tile([128, D], f32, tag="lg", name="ops")
                for pr2 in range(4):
                    nc.tensor.matmul(
                        ops[:],
                        lhsT=r(attnN[pr2][:, qq : qq + 128]),
                        rhs=r(wo_t[pr2]),
                        start=(pr2 == 0),
                        stop=(pr2 == 3),
                    )
                osb = osb_p.tile([128, D], f32, tag="osb", name="osb")
                nc.vector.scalar_tensor_tensor(
                    osb[:],
                    ops[:],
                    1.0,
                    bob,
                    op0=mybir.AluOpType.mult,
                    op1=mybir.AluOpType.add,
                )
                nc.sync.dma_start(d_out[qq : qq + 128, :], osb[:])

        pend = None
        oproj_due = None
        for qc in range(SQ // 512):
            for pr in range(4):
                st = sk_loop(qc, pr)
                if pend is not None:
                    finish(*pend)
                    pend = None
                    if pr == 0 and oproj_due is not None:
                        o_proj(oproj_due)
                        oproj_due = None
                pend = st
            oproj_due = qc
        finish(*pend)
        o_proj(oproj_due)

    nc.compile()
    return nc


def _prep_inputs(query, key, value, mask, wq_w, wq_b, wk_w, wk_b, wv_w, wv_b, wo_w, wo_b,
                 mm_dtype="bf16"):
    import ml_dtypes

    od = {"bf16": ml_dtypes.bfloat16, "f32r": np.float32, "f32": np.float32}[mm_dtype]
    f = lambda a: np.ascontiguousarray(np.asarray(a, dtype=np.float32))
    g = lambda a: np.ascontiguousarray(np.asarray(a).astype(od))
    query, key, value = f(query), f(key), f(value)
    wq_w, wk_w, wv_w, wo_w = f(wq_w), f(wk_w), f(wv_w), f(wo_w)
    mask = np.asarray(mask)

    keeps = [np.flatnonzero(mask[b] == 0) for b in range(B)]
    cnts = [len(k) for k in keeps]
    assert min(cnts) > 0, "all-masked batch not supported"
    s_pad = max(128, ((max(cnts) + 127) // 128) * 128)
    nsk = s_pad // 128

    bq_pp = np.ascontiguousarray(f(wq_b).reshape(4, 128).T)
    bk_pp = np.ascontiguousarray(f(wk_b).reshape(4, 128).T)
    bvb = np.broadcast_to(f(wv_b).reshape(1, D), (128, D))
    bob = np.broadcast_to(f(wo_b).reshape(1, D), (128, D))

    def wchunks(w):
        # [512, 512] -> [128, 4*512]: col block dk holds rows dk*128..dk*128+128
        return w.reshape(4, 128, D).transpose(1, 0, 2).reshape(128, 4 * D)

    def xchunks(xt):
        # [512, S] -> [128, 4*S]
        s = xt.shape[1]
        return xt.reshape(4, 128, s).transpose(1, 0, 2).reshape(128, 4 * s)

    blob_o = np.concatenate([wchunks(f(wo_w)), bob], axis=1)

    common = dict(
        bq_pp=bq_pp, bk_pp=bk_pp,
        blob_o=g(blob_o),
    )
    in_maps = []
    for b in range(B):
        kc = np.zeros((s_pad, D), np.float32)
        kc[: cnts[b]] = key[b][keeps[b]]
        vc = np.zeros((s_pad, D), np.float32)
        vc[: cnts[b]] = value[b][keeps[b]]
        blob_k = g(np.concatenate([wchunks(f(wk_w)), xchunks(kc.T)], axis=1))
        blob_v = g(
            np.concatenate(
                [
                    wchunks(f(wv_w)),
                    xchunks(vc.T),
                    bvb,
                    np.ones((128, 8), np.float32),
                ],
                axis=1,
            )
        )
        mbf = np.zeros(s_pad, np.float32)
        mbf[cnts[b] :] = -1e9
        mbd = np.ascontiguousarray(mbf.reshape(nsk, 128).T)
        for qh in range(2):
            blob_q = g(
                np.concatenate(
                    [wchunks(f(wq_w)), xchunks(query[b, qh * SQ : (qh + 1) * SQ, :].T)],
                    axis=1,
                )
            )
            in_maps.append(
                dict(blob_k=blob_k, blob_q=blob_q, blob_v=blob_v, mb=mbd, **common)
            )
    return s_pad, in_maps


def kernel(**inputs):
    from concourse import bass_utils

    mmd = os.environ.get("BASSK_MMDT", "bf16")
    s_pad, in_maps = _prep_inputs(**inputs, mm_dtype=mmd)
    key = (s_pad, mmd)
    if key not in _BUILT:
        _BUILT[key] = build_bass(s_pad, mm_dtype=key[1])
    nc = _BUILT[key]
    kw = {}
    if os.environ.get("BASSK_TRACE"):
        kw = dict(trace=True, stitch_traces=False)
    res = bass_utils.run_bass_kernel_spmd(nc, in_maps, core_ids=list(range(NCORE)), **kw)
    out = np.empty((B, S, D), np.float32)
    for c in range(NCORE):
        b, qh = c // 2, c % 2
        out[b, qh * SQ : (qh + 1) * SQ, :] = res.results[c]["out"]
    kernel.last_result = res
    return out
